# revision 9
# baseline (speedup 1.0000x reference)
"""Trainium2 Bass kernel for nn_PerturbationGenerator.

Reference computation (B=8, S=4096, H=2048, DH=64, K=256):
  logits = relu(hs @ w1 + b1) @ w2 + b2            # selector MLP, per (b, s)
  y      = logits + gumbel(noise_u)                # gumbel = -log(-log(u))
  sel    = top_k(y, 256) indices (desc by value)
  log_prob = mean(log_softmax(logits)[sel])
  perturbed = hs with rows sel set to 0.0, cast fp16
    (perturb value = TYPE_VALUES[1] * scale = 0.0)

Sharding: pure data parallel over batch; core b handles row b.

Per-core device program (all heavy work on device):
  - stream hs in 32 tiles of (128 s, 2048 h) fp32 [HWDGE]
  - each tile: SWDGE cast-DMA fp32->fp16 straight to the output (RNE cast)
  - each tile: PE-transpose 128x128 chunks (fp32 exact), first MLP layer as
    w1_c.T @ hsT_c accumulated over 16 chunks into PSUM (64, 512) per group
    of 4 s-tiles, relu on ACT, second layer matmul w2.T @ hrelu -> logits
  - y = logits + gumbel placed into the gpsimd-topk layout (16, 3136)
    (vocab padded to 50176 with -1e30)
  - nc.gpsimd.topk: exact top-256 (values + indices, full scan)
  - indirect-DMA scatter of fp16 zero rows over the 256 selected rows
Host: shard/gather, gumbel precompute (jax CPU, matches reference log), final
ordering of indices by device-computed values, log_prob reduction on
device-computed logits (mirrors reference log_softmax/gather/mean).
"""

import os
import numpy as np

B, S, H = 8, 4096, 2048
DH = 64
K = 256
VOCAB = 50176
NPAD = VOCAB // 16  # 3136
NEG = -1e30

_STATE = {}

LAST_EXEC_NS = None


def _build_nc():
    import concourse.bass as bass
    import concourse.bacc as bacc
    import concourse.mybir as mybir
    import concourse.tile as tile
    from concourse import library_config
    from concourse.tile_rust import add_dep_helper

    mdt = mybir.dt
    nc = bacc.Bacc("TRN2", target_bir_lowering=False, debug=False, num_devices=8)

    hs = nc.dram_tensor("hs", [S, H], mdt.float32, kind="ExternalInput")
    gum = nc.dram_tensor("gum", [1, S], mdt.float32, kind="ExternalInput")
    w1_in = nc.dram_tensor("w1_in", [H, DH], mdt.float32, kind="ExternalInput")
    b1_in = nc.dram_tensor("b1_in", [DH, 1], mdt.float32, kind="ExternalInput")
    w2_in = nc.dram_tensor("w2_in", [DH, 1], mdt.float32, kind="ExternalInput")
    ident_in = nc.dram_tensor("ident_in", [128, 128], mdt.float32, kind="ExternalInput")

    pert = nc.dram_tensor("pert", [S, H], mdt.float16, kind="ExternalOutput")
    tko_out = nc.dram_tensor("tko_out", [16, 32], mdt.uint32, kind="ExternalOutput")
    logits_out = nc.dram_tensor("logits_out", [1, S], mdt.float32, kind="ExternalOutput")

    RELU = mybir.ActivationFunctionType.Relu
    COPY = mybir.ActivationFunctionType.Copy

    with tile.TileContext(nc) as tc:
        with (
            tc.tile_pool(name="hsp", bufs=6) as hs_pool,
            tc.tile_pool(name="hstp", bufs=3) as hsT_pool,
            tc.tile_pool(name="small", bufs=1) as const_pool,
            tc.tile_pool(name="ptr", bufs=2, space="PSUM") as psum_tr,
            tc.tile_pool(name="pht", bufs=2, space="PSUM") as psum_hT,
            tc.tile_pool(name="plg", bufs=2, space="PSUM") as psum_lg,
        ):
            # ---- constants / small inputs ----
            nc.gpsimd.load_library(library_config.topk)

            ident = const_pool.tile([128, 128], mdt.float32, tag="ident")
            nc.sync.dma_start(ident[:], ident_in[:])
            # w1 as (128 h-part, 16 chunks, 64 d) so chunk c is w1[c*128:(c+1)*128, :]
            w1_sb = const_pool.tile([128, 16, DH], mdt.float32, tag="w1")
            nc.sync.dma_start(w1_sb[:], w1_in[:].rearrange("(c p) d -> p c d", p=128))
            b1_sb = const_pool.tile([DH, 1], mdt.float32, tag="b1")
            nc.sync.dma_start(b1_sb[:], b1_in[:])
            w2_sb = const_pool.tile([DH, 1], mdt.float32, tag="w2")
            nc.sync.dma_start(w2_sb[:], w2_in[:])
            gum_sb = const_pool.tile([1, S], mdt.float32, tag="gum")
            nc.sync.dma_start(gum_sb[:], gum[:])
            y_row = const_pool.tile([1, S], mdt.float32, tag="y_row")
            zeros16 = const_pool.tile([128, H], mdt.float16, tag="zeros16")
            nc.vector.memset(zeros16[:], 0.0)
            logits_sb = const_pool.tile([1, S], mdt.float32, tag="logits_sb")

            # topk in/out need real SBUF tensors
            y_sb = nc.alloc_sbuf_tensor("y_sb", [16, NPAD], mdt.float32)
            tko = nc.alloc_sbuf_tensor("tko", [16, 32], mdt.uint32)
            nc.vector.memset(y_sb[:], NEG)

            cast_dmas = []

            for g in range(8):
                hs_tiles = []
                for t in range(4):
                    s0 = (g * 4 + t) * 128
                    ht = hs_pool.tile([128, H], mdt.float32, tag="hs")
                    nc.sync.dma_start(ht[:], hs[s0 : s0 + 128, :])
                    cast_dmas.append(nc.gpsimd.dma_start(pert[s0 : s0 + 128, :], ht[:]))
                    hs_tiles.append(ht)

                phT = psum_hT.tile([DH, 512], mdt.float32, tag="phT")
                for c in range(16):
                    ptr = psum_tr.tile([128, 512], mdt.float32, tag="ptr")
                    for t in range(4):
                        nc.tensor.transpose(
                            ptr[:, t * 128 : (t + 1) * 128],
                            hs_tiles[t][:, c * 128 : (c + 1) * 128],
                            ident[:],
                        )
                    hsT = hsT_pool.tile([128, 512], mdt.float32, tag="hsT")
                    nc.vector.tensor_copy(hsT[:], ptr[:])
                    nc.tensor.matmul(
                        phT[:],
                        w1_sb[:, c, :],
                        hsT[:],
                        start=(c == 0),
                        stop=(c == 15),
                    )

                hrelu = hsT_pool.tile([DH, 512], mdt.float32, tag="hrelu")
                nc.scalar.activation(hrelu[:], phT[:], RELU, bias=b1_sb[:, :1], scale=1.0)

                plog = psum_lg.tile([1, 512], mdt.float32, tag="plog")
                nc.tensor.matmul(plog[:], w2_sb[:], hrelu[:], start=True, stop=True)

                nc.scalar.activation(
                    logits_sb[:, g * 512 : (g + 1) * 512], plog[:], COPY
                )
                nc.vector.tensor_tensor(
                    out=y_row[:, g * 512 : (g + 1) * 512],
                    in0=plog[0:1, :],
                    in1=gum_sb[:, g * 512 : (g + 1) * 512],
                    op=mybir.AluOpType.add,
                )

            # ---- tail: place y into topk layout, topk + scatter ----
            # v = s: partition p16 = s // NPAD, col = s % NPAD (only p16 0,1 real)
            nc.sync.dma_start(y_sb[0:1, 0:NPAD], y_row[:, 0:NPAD])
            nc.sync.dma_start(y_sb[1:2, 0 : S - NPAD], y_row[:, NPAD:S])
            nc.gpsimd.topk(tko[:], y_sb[:], tokens=1, vocab_size=VOCAB, k=K)

            idx0 = const_pool.tile([128, 1], mdt.uint32, tag="idx0")
            idx1 = const_pool.tile([128, 1], mdt.uint32, tag="idx1")
            nc.sync.dma_start(idx0[:], tko[0:8, 16:32])
            nc.sync.dma_start(idx1[:], tko[8:16, 16:32])

            for idx in (idx0, idx1):
                sc = nc.gpsimd.indirect_dma_start(
                    out=pert[:],
                    out_offset=bass.IndirectOffsetOnAxis(ap=idx[:, :1], axis=0),
                    in_=zeros16[:],
                    in_offset=None,
                )
                for cd in cast_dmas:
                    add_dep_helper(sc.ins, cd.ins, reason="scatter after cast dma")

            nc.sync.dma_start(tko_out[:], tko[:])
            nc.sync.dma_start(logits_out[:], logits_sb[:])

    nc.compile()
    return nc


def _get_nc():
    if "nc" not in _STATE:
        _STATE["nc"] = _build_nc()
    return _STATE["nc"]


def _gumbel_jax_cpu(noise_u):
    import jax
    import jax.numpy as jnp

    cpu = jax.devices("cpu")[0]
    u = jax.device_put(np.asarray(noise_u, dtype=np.float32), cpu)
    with jax.default_device(cpu):
        g = -jnp.log(-jnp.log(u))
    return np.asarray(g, dtype=np.float32)


def _log_prob_jax_cpu(logits, sel):
    import jax
    import jax.numpy as jnp

    cpu = jax.devices("cpu")[0]
    lg = jax.device_put(np.asarray(logits, dtype=np.float32), cpu)
    si = jax.device_put(np.asarray(sel, dtype=np.int32), cpu)
    with jax.default_device(cpu):
        lp_all = jax.nn.log_softmax(lg, axis=-1)
        lp = jnp.take_along_axis(lp_all, si, axis=1).mean(axis=-1)
    return np.asarray(lp, dtype=np.float32)


def _install_ntff_shim():
    """Provide antenv.axon_hooks if the image lacks it (profiling only)."""
    import sys
    import types
    import contextlib
    import ctypes

    try:
        import antenv.axon_hooks  # noqa: F401
        return
    except ImportError:
        pass

    so_path = "/opt/axon/libaxon_pjrt.so"
    if not os.path.exists(so_path):
        return
    lib = ctypes.CDLL(so_path)
    if not hasattr(lib, "axon_start_nrt_profile"):
        return
    lib.axon_start_nrt_profile.argtypes = [ctypes.POINTER(ctypes.c_int64), ctypes.c_size_t]
    lib.axon_start_nrt_profile.restype = ctypes.c_int64
    lib.axon_stop_nrt_profile.argtypes = [ctypes.c_char_p]
    lib.axon_stop_nrt_profile.restype = ctypes.c_int64

    @contextlib.contextmanager
    def _hook(output_dir, device_ids):
        import jax

        jax.devices()
        if device_ids:
            ids = (ctypes.c_int64 * len(device_ids))(*device_ids)
            rc = lib.axon_start_nrt_profile(ids, len(device_ids))
        else:
            rc = lib.axon_start_nrt_profile(None, 0)
        if rc != 0:
            raise RuntimeError(f"axon_start_nrt_profile rc={rc}")
        try:
            yield
        finally:
            n = lib.axon_stop_nrt_profile(str(output_dir).encode())
            print(f"profile: {n} file(s) written to {output_dir}")

    mod = types.ModuleType("antenv.axon_hooks")
    mod.get_axon_ntff_profile_hook = lambda: _hook
    mod.set_axon_ntff_profile_hook = lambda h: None
    sys.modules["antenv.axon_hooks"] = mod


def kernel(hidden_states, noise_u, w1, b1, w2, b2):
    global LAST_EXEC_NS
    from concourse.bass_utils import run_bass_kernel_spmd

    hidden_states = np.ascontiguousarray(np.asarray(hidden_states, dtype=np.float32))
    noise_u = np.asarray(noise_u, dtype=np.float32)
    w1 = np.ascontiguousarray(np.asarray(w1, dtype=np.float32))
    b1 = np.asarray(b1, dtype=np.float32).reshape(DH, 1)
    w2 = np.ascontiguousarray(np.asarray(w2, dtype=np.float32)).reshape(DH, 1)
    # b2 only shifts logits: top_k order and log_softmax are shift-invariant,
    # so it never affects any output; it is 0.0 in the reference setup.

    gum = _gumbel_jax_cpu(noise_u)  # (B, S)
    ident = np.eye(128, dtype=np.float32)

    in_maps = []
    for b in range(B):
        in_maps.append(
            {
                "hs": hidden_states[b],
                "gum": gum[b : b + 1],
                "w1_in": w1,
                "b1_in": b1,
                "w2_in": w2,
                "ident_in": ident,
            }
        )

    nc = _get_nc()
    trace = bool(int(os.environ.get("BASS_KERNEL_TRACE", "0")))
    if trace:
        _install_ntff_shim()
    res = run_bass_kernel_spmd(
        nc, in_maps, core_ids=list(range(B)), trace=trace,
        trace_cores=[0] if trace else None,
    )
    LAST_EXEC_NS = res.exec_time_ns

    perturbed = np.empty((B, S, H), dtype=np.float16)
    sel_idx = np.empty((B, K), dtype=np.int32)
    logits_all = np.empty((B, S), dtype=np.float32)
    for b in range(B):
        r = res.results[b]
        perturbed[b] = r["pert"]
        tko = r["tko_out"]
        vals = tko[:, :16].copy().view(np.float32).reshape(-1)
        idxs = tko[:, 16:].reshape(-1).astype(np.int64)
        # descending by value; ties -> lower index (matches jax.lax.top_k)
        order = np.lexsort((idxs, -vals.astype(np.float64)))
        sel_idx[b] = idxs[order].astype(np.int32)
        logits_all[b] = r["logits_out"][0]

    perturb_types = np.full((B, K), 1, dtype=np.int32)
    log_prob = _log_prob_jax_cpu(logits_all, sel_idx)
    return perturbed, sel_idx, perturb_types, log_prob


# revision 17
# speedup vs baseline: 1.1463x; 1.1463x over previous
"""Trainium2 Bass kernel for nn_PerturbationGenerator.

Reference computation (B=8, S=4096, H=2048, DH=64, K=256):
  logits = relu(hs @ w1 + b1) @ w2 + b2            # selector MLP, per (b, s)
  y      = logits + gumbel(noise_u)                # gumbel = -log(-log(u))
  sel    = top_k(y, 256) indices (desc by value)
  log_prob = mean(log_softmax(logits)[sel])
  perturbed = hs with rows sel set to 0.0, cast fp16
    (perturb value = TYPE_VALUES[1] * scale = 0.0)

Sharding: pure data parallel over batch; core b handles row b.

Per-core device program (all heavy work on device):
  - stream hs in 32 tiles of (128 s, 2048 h) fp32 [HWDGE]
  - each tile: SWDGE cast-DMA fp32->fp16 straight to the output (RNE cast)
  - each tile: PE-transpose 128x128 chunks (fp32 exact), first MLP layer as
    w1_c.T @ hsT_c accumulated over 16 chunks into PSUM (64, 512) per group
    of 4 s-tiles, relu on ACT, second layer matmul w2.T @ hrelu -> logits
  - y = logits + gumbel placed into the gpsimd-topk layout (16, 3136)
    (vocab padded to 50176 with -1e30)
  - nc.gpsimd.topk: exact top-256 (values + indices, full scan)
  - indirect-DMA scatter of fp16 zero rows over the 256 selected rows
Host: shard/gather, gumbel precompute (jax CPU, matches reference log), final
ordering of indices by device-computed values, log_prob reduction on
device-computed logits (mirrors reference log_softmax/gather/mean).
"""

import os
import numpy as np

B, S, H = 8, 4096, 2048
DH = 64
K = 256
VOCAB = 50176
NPAD = VOCAB // 16  # 3136
NEG = -1e30

_STATE = {}

LAST_EXEC_NS = None


def _build_nc():
    import concourse.bass as bass
    import concourse.bacc as bacc
    import concourse.mybir as mybir
    import concourse.tile as tile
    from concourse import library_config
    from concourse.tile_rust import add_dep_helper

    mdt = mybir.dt
    nc = bacc.Bacc("TRN2", target_bir_lowering=False, debug=False, num_devices=8)

    hs = nc.dram_tensor("hs", [S, H], mdt.float32, kind="ExternalInput")
    gum = nc.dram_tensor("gum", [1, S], mdt.float32, kind="ExternalInput")
    w1_in = nc.dram_tensor("w1_in", [H, DH], mdt.float32, kind="ExternalInput")
    b1_in = nc.dram_tensor("b1_in", [DH, 1], mdt.float32, kind="ExternalInput")
    w2_in = nc.dram_tensor("w2_in", [DH, 1], mdt.float32, kind="ExternalInput")
    ident_in = nc.dram_tensor("ident_in", [128, 128], mdt.float32, kind="ExternalInput")

    pert = nc.dram_tensor("pert", [S, H], mdt.float16, kind="ExternalOutput")
    tko_out = nc.dram_tensor("tko_out", [16, 32], mdt.uint32, kind="ExternalOutput")
    logits_out = nc.dram_tensor("logits_out", [1, S], mdt.float32, kind="ExternalOutput")

    RELU = mybir.ActivationFunctionType.Relu
    COPY = mybir.ActivationFunctionType.Copy

    with tile.TileContext(nc) as tc:
        with (
            tc.tile_pool(name="hsp", bufs=12) as hs_pool,
            tc.tile_pool(name="hstp", bufs=3) as hsT_pool,
            tc.tile_pool(name="small", bufs=1) as const_pool,
            tc.tile_pool(name="ptr", bufs=2, space="PSUM") as psum_tr,
            tc.tile_pool(name="pht", bufs=2, space="PSUM") as psum_hT,
            tc.tile_pool(name="plg", bufs=2, space="PSUM") as psum_lg,
        ):
            # ---- constants / small inputs ----
            nc.gpsimd.load_library(library_config.topk)

            ident = const_pool.tile([128, 128], mdt.float32, tag="ident")
            nc.sync.dma_start(ident[:], ident_in[:])
            # w1 as (128 h-part, 16 chunks, 64 d) so chunk c is w1[c*128:(c+1)*128, :]
            w1_sb = const_pool.tile([128, 16, DH], mdt.float32, tag="w1")
            nc.sync.dma_start(w1_sb[:], w1_in[:].rearrange("(c p) d -> p c d", p=128))
            b1_sb = const_pool.tile([DH, 1], mdt.float32, tag="b1")
            nc.sync.dma_start(b1_sb[:], b1_in[:])
            w2_sb = const_pool.tile([DH, 1], mdt.float32, tag="w2")
            nc.sync.dma_start(w2_sb[:], w2_in[:])
            gum_sb = const_pool.tile([1, S], mdt.float32, tag="gum")
            nc.sync.dma_start(gum_sb[:], gum[:])
            y_row = const_pool.tile([1, S], mdt.float32, tag="y_row")
            zeros16 = const_pool.tile([128, H], mdt.float16, tag="zeros16")
            nc.vector.memset(zeros16[:], 0.0)
            logits_sb = const_pool.tile([1, S], mdt.float32, tag="logits_sb")

            # topk in/out need real SBUF tensors
            y_sb = nc.alloc_sbuf_tensor("y_sb", [16, NPAD], mdt.float32)
            tko = nc.alloc_sbuf_tensor("tko", [16, 32], mdt.uint32)
            nc.vector.memset(y_sb[:], NEG)

            cast_dmas = []

            for g in range(8):
                hs_tiles = []
                for t in range(4):
                    s0 = (g * 4 + t) * 128
                    ht = hs_pool.tile([128, H], mdt.float32, tag="hs")
                    nc.sync.dma_start(ht[:], hs[s0 : s0 + 128, :])
                    cast_dmas.append(nc.gpsimd.dma_start(pert[s0 : s0 + 128, :], ht[:]))
                    hs_tiles.append(ht)

                phT = psum_hT.tile([DH, 512], mdt.float32, tag="phT")
                for c in range(16):
                    ptr = psum_tr.tile([128, 512], mdt.float32, tag="ptr")
                    for t in range(4):
                        nc.tensor.transpose(
                            ptr[:, t * 128 : (t + 1) * 128],
                            hs_tiles[t][:, c * 128 : (c + 1) * 128],
                            ident[:],
                        )
                    hsT = hsT_pool.tile([128, 512], mdt.float32, tag="hsT")
                    if c % 2 == 0:
                        nc.vector.tensor_copy(hsT[:], ptr[:])
                    else:
                        nc.scalar.activation(hsT[:], ptr[:], COPY)
                    nc.tensor.matmul(
                        phT[:],
                        w1_sb[:, c, :],
                        hsT[:],
                        start=(c == 0),
                        stop=(c == 15),
                    )

                hrelu = hsT_pool.tile([DH, 512], mdt.float32, tag="hrelu")
                nc.scalar.activation(hrelu[:], phT[:], RELU, bias=b1_sb[:, :1], scale=1.0)

                plog = psum_lg.tile([1, 512], mdt.float32, tag="plog")
                nc.tensor.matmul(plog[:], w2_sb[:], hrelu[:], start=True, stop=True)

                nc.scalar.activation(
                    logits_sb[:, g * 512 : (g + 1) * 512], plog[:], COPY
                )
                nc.vector.tensor_tensor(
                    out=y_row[:, g * 512 : (g + 1) * 512],
                    in0=plog[0:1, :],
                    in1=gum_sb[:, g * 512 : (g + 1) * 512],
                    op=mybir.AluOpType.add,
                )

            # ---- tail: place y into topk layout, topk + scatter ----
            # v = s blocked mapping: partition p16 = s // NPAD, col = s % NPAD.
            # (the topk ucode miscounts when >2 partitions carry real data,
            # so keep everything in partitions 0-1; scan cost is fixed anyway)
            nc.sync.dma_start(y_sb[0:1, 0:NPAD], y_row[:, 0:NPAD])
            nc.sync.dma_start(y_sb[1:2, 0 : S - NPAD], y_row[:, NPAD:S])
            nc.gpsimd.topk(tko[:], y_sb[:], tokens=1, vocab_size=VOCAB, k=K)

            # blocked mapping means v == s directly: NPAD > 4096 - NPAD, so
            # p16 is 0 or 1 and v = p16 * NPAD + c = s. No conversion needed.
            idx0 = const_pool.tile([128, 1], mdt.uint32, tag="idx0")
            idx1 = const_pool.tile([128, 1], mdt.uint32, tag="idx1")
            nc.sync.dma_start(idx0[:], tko[0:8, 16:32])
            nc.sync.dma_start(idx1[:], tko[8:16, 16:32])

            for idx in (idx0, idx1):
                sc = nc.gpsimd.indirect_dma_start(
                    out=pert[:],
                    out_offset=bass.IndirectOffsetOnAxis(ap=idx[:, :1], axis=0),
                    in_=zeros16[:],
                    in_offset=None,
                )
                for cd in cast_dmas:
                    add_dep_helper(sc.ins, cd.ins, reason="scatter after cast dma")

            nc.sync.dma_start(tko_out[:], tko[:])
            nc.sync.dma_start(logits_out[:], logits_sb[:])

    nc.compile()
    return nc


def _get_nc():
    if "nc" not in _STATE:
        _STATE["nc"] = _build_nc()
    return _STATE["nc"]


def _gumbel_jax_cpu(noise_u):
    import jax
    import jax.numpy as jnp

    cpu = jax.devices("cpu")[0]
    u = jax.device_put(np.asarray(noise_u, dtype=np.float32), cpu)
    with jax.default_device(cpu):
        g = -jnp.log(-jnp.log(u))
    return np.asarray(g, dtype=np.float32)


def _log_prob_jax_cpu(logits, sel):
    import jax
    import jax.numpy as jnp

    cpu = jax.devices("cpu")[0]
    lg = jax.device_put(np.asarray(logits, dtype=np.float32), cpu)
    si = jax.device_put(np.asarray(sel, dtype=np.int32), cpu)
    with jax.default_device(cpu):
        lp_all = jax.nn.log_softmax(lg, axis=-1)
        lp = jnp.take_along_axis(lp_all, si, axis=1).mean(axis=-1)
    return np.asarray(lp, dtype=np.float32)


def _install_ntff_shim():
    """Provide antenv.axon_hooks if the image lacks it (profiling only)."""
    import sys
    import types
    import contextlib
    import ctypes

    try:
        import antenv.axon_hooks  # noqa: F401
        return
    except ImportError:
        pass

    so_path = "/opt/axon/libaxon_pjrt.so"
    if not os.path.exists(so_path):
        return
    lib = ctypes.CDLL(so_path)
    if not hasattr(lib, "axon_start_nrt_profile"):
        return
    lib.axon_start_nrt_profile.argtypes = [ctypes.POINTER(ctypes.c_int64), ctypes.c_size_t]
    lib.axon_start_nrt_profile.restype = ctypes.c_int64
    lib.axon_stop_nrt_profile.argtypes = [ctypes.c_char_p]
    lib.axon_stop_nrt_profile.restype = ctypes.c_int64

    @contextlib.contextmanager
    def _hook(output_dir, device_ids):
        import jax

        jax.devices()
        if device_ids:
            ids = (ctypes.c_int64 * len(device_ids))(*device_ids)
            rc = lib.axon_start_nrt_profile(ids, len(device_ids))
        else:
            rc = lib.axon_start_nrt_profile(None, 0)
        if rc != 0:
            raise RuntimeError(f"axon_start_nrt_profile rc={rc}")
        try:
            yield
        finally:
            n = lib.axon_stop_nrt_profile(str(output_dir).encode())
            print(f"profile: {n} file(s) written to {output_dir}")

    mod = types.ModuleType("antenv.axon_hooks")
    mod.get_axon_ntff_profile_hook = lambda: _hook
    mod.set_axon_ntff_profile_hook = lambda h: None
    sys.modules["antenv.axon_hooks"] = mod


def kernel(hidden_states, noise_u, w1, b1, w2, b2):
    global LAST_EXEC_NS
    from concourse.bass_utils import run_bass_kernel_spmd

    hidden_states = np.ascontiguousarray(np.asarray(hidden_states, dtype=np.float32))
    noise_u = np.asarray(noise_u, dtype=np.float32)
    w1 = np.ascontiguousarray(np.asarray(w1, dtype=np.float32))
    b1 = np.asarray(b1, dtype=np.float32).reshape(DH, 1)
    w2 = np.ascontiguousarray(np.asarray(w2, dtype=np.float32)).reshape(DH, 1)
    # b2 only shifts logits: top_k order and log_softmax are shift-invariant,
    # so it never affects any output; it is 0.0 in the reference setup.

    gum = _gumbel_jax_cpu(noise_u)  # (B, S)
    ident = np.eye(128, dtype=np.float32)

    in_maps = []
    for b in range(B):
        in_maps.append(
            {
                "hs": hidden_states[b],
                "gum": gum[b : b + 1],
                "w1_in": w1,
                "b1_in": b1,
                "w2_in": w2,
                "ident_in": ident,
            }
        )

    nc = _get_nc()
    trace = bool(int(os.environ.get("BASS_KERNEL_TRACE", "0")))
    if trace:
        _install_ntff_shim()
    res = run_bass_kernel_spmd(
        nc, in_maps, core_ids=list(range(B)), trace=trace,
        trace_cores=[0] if trace else None,
    )
    LAST_EXEC_NS = res.exec_time_ns

    perturbed = np.empty((B, S, H), dtype=np.float16)
    sel_idx = np.empty((B, K), dtype=np.int32)
    logits_all = np.empty((B, S), dtype=np.float32)
    for b in range(B):
        r = res.results[b]
        perturbed[b] = r["pert"]
        tko = r["tko_out"]
        vals = tko[:, :16].copy().view(np.float32).reshape(-1)
        idxs = tko[:, 16:].reshape(-1).astype(np.int64)  # v == s (blocked layout)
        # descending by value; ties -> lower index (matches jax.lax.top_k)
        order = np.lexsort((idxs, -vals.astype(np.float64)))
        sel_idx[b] = idxs[order].astype(np.int32)
        logits_all[b] = r["logits_out"][0]

    perturb_types = np.full((B, K), 1, dtype=np.int32)
    log_prob = _log_prob_jax_cpu(logits_all, sel_idx)
    return perturbed, sel_idx, perturb_types, log_prob


# revision 18
# speedup vs baseline: 1.2790x; 1.1158x over previous
"""Trainium2 Bass kernel for nn_PerturbationGenerator.

Reference computation (B=8, S=4096, H=2048, DH=64, K=256):
  logits = relu(hs @ w1 + b1) @ w2 + b2            # selector MLP, per (b, s)
  y      = logits + gumbel(noise_u)                # gumbel = -log(-log(u))
  sel    = top_k(y, 256) indices (desc by value)
  log_prob = mean(log_softmax(logits)[sel])
  perturbed = hs with rows sel set to 0.0, cast fp16
    (perturb value = TYPE_VALUES[1] * scale = 0.0)

Sharding: pure data parallel over batch; core b handles row b.

Per-core device program (all heavy work on device):
  - stream hs in 32 tiles of (128 s, 2048 h) fp32 [HWDGE]
  - each tile: SWDGE cast-DMA fp32->fp16 straight to the output (RNE cast)
  - each tile: PE-transpose 128x128 chunks (fp32 exact), first MLP layer as
    w1_c.T @ hsT_c accumulated over 16 chunks into PSUM (64, 512) per group
    of 4 s-tiles, relu on ACT, second layer matmul w2.T @ hrelu -> logits
  - y = logits + gumbel placed into the gpsimd-topk layout (16, 3136)
    (vocab padded to 50176 with -1e30)
  - nc.gpsimd.topk: exact top-256 (values + indices, full scan)
  - indirect-DMA scatter of fp16 zero rows over the 256 selected rows
Host: shard/gather, gumbel precompute (jax CPU, matches reference log), final
ordering of indices by device-computed values, log_prob reduction on
device-computed logits (mirrors reference log_softmax/gather/mean).
"""

import os
import numpy as np

B, S, H = 8, 4096, 2048
DH = 64
K = 256
VOCAB = 50176
NPAD = VOCAB // 16  # 3136
NEG = -1e30

_STATE = {}

LAST_EXEC_NS = None


def _build_nc():
    import concourse.bass as bass
    import concourse.bacc as bacc
    import concourse.mybir as mybir
    import concourse.tile as tile
    from concourse import library_config
    from concourse.tile_rust import add_dep_helper

    mdt = mybir.dt
    nc = bacc.Bacc("TRN2", target_bir_lowering=False, debug=False, num_devices=8)

    hs = nc.dram_tensor("hs", [S, H], mdt.float32, kind="ExternalInput")
    gum = nc.dram_tensor("gum", [1, S], mdt.float32, kind="ExternalInput")
    w1_in = nc.dram_tensor("w1_in", [H, DH], mdt.float32, kind="ExternalInput")
    b1_in = nc.dram_tensor("b1_in", [DH, 1], mdt.float32, kind="ExternalInput")
    w2_in = nc.dram_tensor("w2_in", [DH, 1], mdt.float32, kind="ExternalInput")
    ident_in = nc.dram_tensor("ident_in", [128, 128], mdt.float32, kind="ExternalInput")

    pert = nc.dram_tensor("pert", [S, H], mdt.float16, kind="ExternalOutput")
    tko_out = nc.dram_tensor("tko_out", [16, 32], mdt.uint32, kind="ExternalOutput")
    logits_out = nc.dram_tensor("logits_out", [1, S], mdt.float32, kind="ExternalOutput")

    RELU = mybir.ActivationFunctionType.Relu
    COPY = mybir.ActivationFunctionType.Copy

    with tile.TileContext(nc) as tc:
        with (
            tc.tile_pool(name="hsp", bufs=12) as hs_pool,
            tc.tile_pool(name="hstp", bufs=4) as hsT_pool,
            tc.tile_pool(name="small", bufs=1) as const_pool,
            tc.tile_pool(name="ptr", bufs=3, space="PSUM") as psum_tr,
            tc.tile_pool(name="pht", bufs=2, space="PSUM") as psum_hT,
            tc.tile_pool(name="plg", bufs=2, space="PSUM") as psum_lg,
        ):
            # ---- constants / small inputs ----
            nc.gpsimd.load_library(library_config.topk)

            ident = const_pool.tile([128, 128], mdt.float32, tag="ident")
            nc.sync.dma_start(ident[:], ident_in[:])
            # w1 as (128 h-part, 16 chunks, 64 d) so chunk c is w1[c*128:(c+1)*128, :]
            w1_sb = const_pool.tile([128, 16, DH], mdt.float32, tag="w1")
            nc.sync.dma_start(w1_sb[:], w1_in[:].rearrange("(c p) d -> p c d", p=128))
            b1_sb = const_pool.tile([DH, 1], mdt.float32, tag="b1")
            nc.sync.dma_start(b1_sb[:], b1_in[:])
            w2_sb = const_pool.tile([DH, 1], mdt.float32, tag="w2")
            nc.sync.dma_start(w2_sb[:], w2_in[:])
            gum_sb = const_pool.tile([1, S], mdt.float32, tag="gum")
            nc.sync.dma_start(gum_sb[:], gum[:])
            y_row = const_pool.tile([1, S], mdt.float32, tag="y_row")
            zeros16 = const_pool.tile([128, H], mdt.float16, tag="zeros16")
            nc.vector.memset(zeros16[:], 0.0)
            logits_sb = const_pool.tile([1, S], mdt.float32, tag="logits_sb")

            # topk in/out need real SBUF tensors
            y_sb = nc.alloc_sbuf_tensor("y_sb", [16, NPAD], mdt.float32)
            tko = nc.alloc_sbuf_tensor("tko", [16, 32], mdt.uint32)
            nc.vector.memset(y_sb[:], NEG)

            cast_dmas = []

            for g in range(8):
                hs_tiles = []
                for t in range(4):
                    s0 = (g * 4 + t) * 128
                    ht = hs_pool.tile([128, H], mdt.float32, tag="hs")
                    nc.sync.dma_start(ht[:], hs[s0 : s0 + 128, :])
                    cast_dmas.append(nc.gpsimd.dma_start(pert[s0 : s0 + 128, :], ht[:]))
                    hs_tiles.append(ht)

                phT = psum_hT.tile([DH, 512], mdt.float32, tag="phT")
                for c in range(16):
                    ptr = psum_tr.tile([128, 512], mdt.float32, tag="ptr")
                    for t in range(4):
                        nc.tensor.transpose(
                            ptr[:, t * 128 : (t + 1) * 128],
                            hs_tiles[t][:, c * 128 : (c + 1) * 128],
                            ident[:],
                        )
                    hsT = hsT_pool.tile([128, 512], mdt.float32, tag="hsT")
                    if c % 2 == 0:
                        nc.vector.tensor_copy(hsT[:], ptr[:])
                    else:
                        nc.scalar.activation(hsT[:], ptr[:], COPY)
                    nc.tensor.matmul(
                        phT[:],
                        w1_sb[:, c, :],
                        hsT[:],
                        start=(c == 0),
                        stop=(c == 15),
                    )

                hrelu = hsT_pool.tile([DH, 512], mdt.float32, tag="hrelu")
                nc.scalar.activation(hrelu[:], phT[:], RELU, bias=b1_sb[:, :1], scale=1.0)

                plog = psum_lg.tile([1, 512], mdt.float32, tag="plog")
                nc.tensor.matmul(plog[:], w2_sb[:], hrelu[:], start=True, stop=True)

                nc.scalar.activation(
                    logits_sb[:, g * 512 : (g + 1) * 512], plog[:], COPY
                )
                nc.vector.tensor_tensor(
                    out=y_row[:, g * 512 : (g + 1) * 512],
                    in0=plog[0:1, :],
                    in1=gum_sb[:, g * 512 : (g + 1) * 512],
                    op=mybir.AluOpType.add,
                )

            # ---- tail: place y into topk layout, topk + scatter ----
            # v = s blocked mapping: partition p16 = s // NPAD, col = s % NPAD.
            # (the topk ucode miscounts when >2 partitions carry real data,
            # so keep everything in partitions 0-1; scan cost is fixed anyway)
            nc.sync.dma_start(y_sb[0:1, 0:NPAD], y_row[:, 0:NPAD])
            nc.sync.dma_start(y_sb[1:2, 0 : S - NPAD], y_row[:, NPAD:S])
            nc.gpsimd.topk(tko[:], y_sb[:], tokens=1, vocab_size=VOCAB, k=K)

            # blocked mapping means v == s directly: NPAD > 4096 - NPAD, so
            # p16 is 0 or 1 and v = p16 * NPAD + c = s. No conversion needed.
            idx0 = const_pool.tile([128, 1], mdt.uint32, tag="idx0")
            idx1 = const_pool.tile([128, 1], mdt.uint32, tag="idx1")
            nc.sync.dma_start(idx0[:], tko[0:8, 16:32])
            nc.sync.dma_start(idx1[:], tko[8:16, 16:32])

            for idx in (idx0, idx1):
                sc = nc.gpsimd.indirect_dma_start(
                    out=pert[:],
                    out_offset=bass.IndirectOffsetOnAxis(ap=idx[:, :1], axis=0),
                    in_=zeros16[:],
                    in_offset=None,
                )
                for cd in cast_dmas:
                    add_dep_helper(sc.ins, cd.ins, reason="scatter after cast dma")

            nc.sync.dma_start(tko_out[:], tko[:])
            nc.sync.dma_start(logits_out[:], logits_sb[:])

    nc.compile()
    return nc


def _get_nc():
    if "nc" not in _STATE:
        _STATE["nc"] = _build_nc()
    return _STATE["nc"]


def _gumbel_jax_cpu(noise_u):
    import jax
    import jax.numpy as jnp

    cpu = jax.devices("cpu")[0]
    u = jax.device_put(np.asarray(noise_u, dtype=np.float32), cpu)
    with jax.default_device(cpu):
        g = -jnp.log(-jnp.log(u))
    return np.asarray(g, dtype=np.float32)


def _log_prob_jax_cpu(logits, sel):
    import jax
    import jax.numpy as jnp

    cpu = jax.devices("cpu")[0]
    lg = jax.device_put(np.asarray(logits, dtype=np.float32), cpu)
    si = jax.device_put(np.asarray(sel, dtype=np.int32), cpu)
    with jax.default_device(cpu):
        lp_all = jax.nn.log_softmax(lg, axis=-1)
        lp = jnp.take_along_axis(lp_all, si, axis=1).mean(axis=-1)
    return np.asarray(lp, dtype=np.float32)


def _install_ntff_shim():
    """Provide antenv.axon_hooks if the image lacks it (profiling only)."""
    import sys
    import types
    import contextlib
    import ctypes

    try:
        import antenv.axon_hooks  # noqa: F401
        return
    except ImportError:
        pass

    so_path = "/opt/axon/libaxon_pjrt.so"
    if not os.path.exists(so_path):
        return
    lib = ctypes.CDLL(so_path)
    if not hasattr(lib, "axon_start_nrt_profile"):
        return
    lib.axon_start_nrt_profile.argtypes = [ctypes.POINTER(ctypes.c_int64), ctypes.c_size_t]
    lib.axon_start_nrt_profile.restype = ctypes.c_int64
    lib.axon_stop_nrt_profile.argtypes = [ctypes.c_char_p]
    lib.axon_stop_nrt_profile.restype = ctypes.c_int64

    @contextlib.contextmanager
    def _hook(output_dir, device_ids):
        import jax

        jax.devices()
        if device_ids:
            ids = (ctypes.c_int64 * len(device_ids))(*device_ids)
            rc = lib.axon_start_nrt_profile(ids, len(device_ids))
        else:
            rc = lib.axon_start_nrt_profile(None, 0)
        if rc != 0:
            raise RuntimeError(f"axon_start_nrt_profile rc={rc}")
        try:
            yield
        finally:
            n = lib.axon_stop_nrt_profile(str(output_dir).encode())
            print(f"profile: {n} file(s) written to {output_dir}")

    mod = types.ModuleType("antenv.axon_hooks")
    mod.get_axon_ntff_profile_hook = lambda: _hook
    mod.set_axon_ntff_profile_hook = lambda h: None
    sys.modules["antenv.axon_hooks"] = mod


def kernel(hidden_states, noise_u, w1, b1, w2, b2):
    global LAST_EXEC_NS
    from concourse.bass_utils import run_bass_kernel_spmd

    hidden_states = np.ascontiguousarray(np.asarray(hidden_states, dtype=np.float32))
    noise_u = np.asarray(noise_u, dtype=np.float32)
    w1 = np.ascontiguousarray(np.asarray(w1, dtype=np.float32))
    b1 = np.asarray(b1, dtype=np.float32).reshape(DH, 1)
    w2 = np.ascontiguousarray(np.asarray(w2, dtype=np.float32)).reshape(DH, 1)
    # b2 only shifts logits: top_k order and log_softmax are shift-invariant,
    # so it never affects any output; it is 0.0 in the reference setup.

    gum = _gumbel_jax_cpu(noise_u)  # (B, S)
    ident = np.eye(128, dtype=np.float32)

    in_maps = []
    for b in range(B):
        in_maps.append(
            {
                "hs": hidden_states[b],
                "gum": gum[b : b + 1],
                "w1_in": w1,
                "b1_in": b1,
                "w2_in": w2,
                "ident_in": ident,
            }
        )

    nc = _get_nc()
    trace = bool(int(os.environ.get("BASS_KERNEL_TRACE", "0")))
    if trace:
        _install_ntff_shim()
    res = run_bass_kernel_spmd(
        nc, in_maps, core_ids=list(range(B)), trace=trace,
        trace_cores=[0] if trace else None,
    )
    LAST_EXEC_NS = res.exec_time_ns

    perturbed = np.empty((B, S, H), dtype=np.float16)
    sel_idx = np.empty((B, K), dtype=np.int32)
    logits_all = np.empty((B, S), dtype=np.float32)
    for b in range(B):
        r = res.results[b]
        perturbed[b] = r["pert"]
        tko = r["tko_out"]
        vals = tko[:, :16].copy().view(np.float32).reshape(-1)
        idxs = tko[:, 16:].reshape(-1).astype(np.int64)  # v == s (blocked layout)
        # descending by value; ties -> lower index (matches jax.lax.top_k)
        order = np.lexsort((idxs, -vals.astype(np.float64)))
        sel_idx[b] = idxs[order].astype(np.int32)
        logits_all[b] = r["logits_out"][0]

    perturb_types = np.full((B, K), 1, dtype=np.int32)
    log_prob = _log_prob_jax_cpu(logits_all, sel_idx)
    return perturbed, sel_idx, perturb_types, log_prob
